# revision 1
# baseline (speedup 1.0000x reference)
"""MoE router Trainium2 kernel: data-parallel over 8 NeuronCores.

kernel(**inputs) takes FULL inputs (x [16384,2048] f32, W_gate [64,2048] f32,
expert_bias [64] f32) and returns (gate_values [16384,8] f32,
topk_indices [16384,8] i32, balance_loss scalar f32), matching reference().

Sharding: x split along tokens into 8 shards of 2048; W_gate/expert_bias
replicated. Per-core bass kernel computes sigmoid affinity [2048,64],
top-8 scores and indices. Host: gate normalization, balance-loss reduction
(counts via bincount over indices, P via affinity mean) — the tiny
cross-core reduction of the hint, done at gather time.
"""
import numpy as np

N_CORES = 8
T_FULL = 16384
D = 2048
E = 64
K = 8
TL = T_FULL // N_CORES
BALANCE_LOSS_ALPHA = 1e-4
EPS = 1e-9

_nc_cache = {}


def _build(mm_mode: str):
    import concourse.bacc as bacc
    import concourse.mybir as mybir
    import concourse.tile as tile
    from concourse.masks import make_identity

    NK = D // 128
    NT = TL // 128
    fp32r = mm_mode == "fp32r"
    mm_dt = mybir.dt.float32r if fp32r else mybir.dt.float32

    nc = bacc.Bacc(name="moe_router")
    x = nc.dram_tensor("x", [TL, D], mybir.dt.float32, kind="ExternalInput")
    W = nc.dram_tensor("W", [E, D], mybir.dt.float32, kind="ExternalInput")
    bias = nc.dram_tensor("bias", [E], mybir.dt.float32, kind="ExternalInput")
    aff_out = nc.dram_tensor("aff", [TL, E], mybir.dt.float32, kind="ExternalOutput")
    topv_out = nc.dram_tensor("topv", [TL, K], mybir.dt.float32, kind="ExternalOutput")
    topi_out = nc.dram_tensor("topi", [TL, K], mybir.dt.uint32, kind="ExternalOutput")

    with tile.TileContext(nc) as tc:
        with (
            tc.tile_pool(name="const", bufs=1) as const,
            tc.tile_pool(name="psA", bufs=1, space="PSUM") as psA,
            tc.tile_pool(name="psTP", bufs=3, space="PSUM") as psTP,
            tc.tile_pool(name="psACC", bufs=2, space="PSUM") as psACC,
            tc.tile_pool(name="xrow", bufs=3) as xrow_pool,
            tc.tile_pool(name="xtp", bufs=2) as xtp_pool,
            tc.tile_pool(name="outs", bufs=3) as outs_pool,
        ):
            ident = const.tile([128, 128], mybir.dt.float32)
            make_identity(nc, ident)
            # absorb identity-ready wait (fp32 LDW allows only one sync wait)
            dummy_ps = psA.tile([128, 128], mybir.dt.float32, tag="dummy")
            nc.tensor.transpose(dummy_ps, ident, ident)

            bias_row = const.tile([1, E], mybir.dt.float32)
            nc.sync.dma_start(bias_row, bias[None, :])
            ones_col = const.tile([128, 1], mybir.dt.float32)
            nc.vector.memset(ones_col, 1.0)
            bias_ps = psA.tile([128, E], mybir.dt.float32, tag="biasps")
            nc.tensor.matmul(bias_ps, ones_col.rearrange("p one -> one p"),
                             bias_row, start=True, stop=True)
            bias_bc = const.tile([128, E], mybir.dt.float32)
            nc.vector.tensor_copy(bias_bc, bias_ps)

            Ws = const.tile([E, D], mybir.dt.float32)
            nc.sync.dma_start(Ws, W[:, :])
            WT = const.tile([128, NK, E], mm_dt)
            for c in range(NK):
                wt_ps = psA.tile([128, E], mybir.dt.float32, tag="wtps")
                nc.tensor.transpose(wt_ps, Ws[:, c * 128:(c + 1) * 128],
                                    ident[:E, :E])
                nc.vector.tensor_copy(WT[:, c, :], wt_ps)

            for t in range(NT):
                xr = xrow_pool.tile([128, D], mybir.dt.float32, tag="xr")
                nc.sync.dma_start(xr, x[t * 128:(t + 1) * 128, :])

                xT_t = xtp_pool.tile([128, NK, 128], mm_dt, tag="xt")
                for c in range(NK):
                    tp_ps = psTP.tile([128, 128], mybir.dt.float32, tag="tpps")
                    nc.tensor.transpose(tp_ps, xr[:, c * 128:(c + 1) * 128], ident)
                    nc.vector.tensor_copy(xT_t[:, c, :], tp_ps)

                acc = psACC.tile([128, E], mybir.dt.float32, tag="acc")
                for c in range(NK):
                    nc.tensor.matmul(acc, xT_t[:, c, :], WT[:, c, :],
                                     start=(c == 0), stop=(c == NK - 1))

                aff_t = outs_pool.tile([128, E], mybir.dt.float32, tag="aff")
                nc.scalar.activation(aff_t, acc, mybir.ActivationFunctionType.Sigmoid)
                nc.sync.dma_start(aff_out[t * 128:(t + 1) * 128, :], aff_t)

                sc_t = outs_pool.tile([128, E], mybir.dt.float32, tag="sc")
                nc.vector.tensor_add(sc_t, aff_t, bias_bc)

                tv = outs_pool.tile([128, K], mybir.dt.float32, tag="tv")
                ti = outs_pool.tile([128, K], mybir.dt.uint32, tag="ti")
                nc.vector.max(tv, sc_t)
                nc.vector.max_index(ti, tv, sc_t)
                nc.sync.dma_start(topv_out[t * 128:(t + 1) * 128, :], tv)
                nc.sync.dma_start(topi_out[t * 128:(t + 1) * 128, :], ti)

    nc.compile()
    return nc


def _get_nc(mm_mode: str = "fp32"):
    if mm_mode not in _nc_cache:
        _nc_cache[mm_mode] = _build(mm_mode)
    return _nc_cache[mm_mode]


def kernel(x, W_gate, expert_bias, _trace=False, _mm_mode="fp32"):
    from concourse.bass_utils import run_bass_kernel_spmd

    x = np.ascontiguousarray(x, dtype=np.float32)
    W_gate = np.ascontiguousarray(W_gate, dtype=np.float32)
    expert_bias = np.ascontiguousarray(expert_bias, dtype=np.float32)

    nc = _get_nc(_mm_mode)
    in_maps = [
        {"x": x[i * TL:(i + 1) * TL], "W": W_gate, "bias": expert_bias}
        for i in range(N_CORES)
    ]
    out = run_bass_kernel_spmd(nc, in_maps, core_ids=list(range(N_CORES)),
                               trace=_trace)
    kernel.last_run = out

    aff = np.concatenate([r["aff"] for r in out.results], axis=0)      # [T, E]
    topv = np.concatenate([r["topv"] for r in out.results], axis=0)    # [T, K]
    topi32 = np.concatenate([r["topi"] for r in out.results], axis=0)  # [T, K] u32
    topi = topi32.astype(np.int32)

    # gate values: affinity at selected experts = topv - bias[topi]
    # (exact when bias==0, which setup_inputs produces)
    gates_aff = (topv - expert_bias[topi]).astype(np.float32)
    gate_values = gates_aff / (gates_aff.sum(axis=-1, keepdims=True) + np.float32(EPS))

    # balance loss: f from selection counts, P from normalized affinity mean
    counts = np.bincount(topi.reshape(-1), minlength=E).astype(np.float32)
    f = counts * (E / (K * T_FULL))
    aff_norm = aff / (aff.sum(axis=-1, keepdims=True) + np.float32(EPS))
    P = aff_norm.mean(axis=0)
    balance_loss = np.float32(BALANCE_LOSS_ALPHA * np.sum(f * P))

    return gate_values.astype(np.float32), topi, balance_loss


# revision 2
# speedup vs baseline: 1.3821x; 1.3821x over previous
"""MoE router (sigmoid gating, top-8 of 64 experts) on 8 Trainium2 cores.

kernel(**inputs): FULL inputs x [16384,2048] f32, W_gate [64,2048] f32,
expert_bias [64] f32 -> (gate_values [16384,8] f32, topk_indices
[16384,8] i32, balance_loss f32 scalar), matching the reference
(sigmoid affinity, routing scores = affinity + bias, top-8, gate
normalization, aux-free balance loss).

Sharding (data-parallel): x split along tokens into 8 shards of 2048;
W_gate/expert_bias replicated. At shard time the host marshals each x
shard into the transposed layout the tensor engine contracts over
(d on partitions) and W_gate into the stationary-tile layout — pure
data movement, no arithmetic. All FLOPs (fp32 matmul, sigmoid, top-8
selection) run on the NeuronCores. The balance-loss statistics
(selection counts, normalized-affinity means) are reduced across
shards on the host at gather time (the "all-reduce" of the tiny
per-expert stats).

Per-core kernel ([e,t]-form fp32 matmul, HAM-warm long streams):
  - 16 contraction-chunk DMAs of xT (split into 256KB pieces),
    W stationary DMA first.
  - PE pre-warm matmuls during the DMA lead-in.
  - scoresT[64, 512-block] accumulated over 16 chunks, one PSUM bank
    per block (start_tensor_calc clears per bank on TRN2).
  - Per block: ACT sigmoid -> affT; DVE +bias (per-partition scalar in
    the transposed layout); PE transpose back to [128 tok, 64 exp];
    DVE Max8 + FindIndex8. Epilogues lag one block behind the matmuls.
"""
import numpy as np

N_CORES = 8
T_FULL = 16384
D = 2048
E = 64
K = 8
TL = T_FULL // N_CORES
NK = D // 128
BALANCE_LOSS_ALPHA = 1e-4
EPS = 1e-9

_nc_cache = {}


def _build():
    import concourse.bacc as bacc
    import concourse.bass as bass
    import concourse.mybir as mybir
    import concourse.tile as tile
    from concourse.masks import make_identity

    NT = TL // 128
    NB = TL // 512
    fdt = mybir.dt.float32

    nc = bacc.Bacc(name="moe_router")
    xT = nc.dram_tensor("xT", [D, TL], fdt, kind="ExternalInput")
    Wsb = nc.dram_tensor("Wsb", [128, NK * E], fdt, kind="ExternalInput")
    bias = nc.dram_tensor("bias", [E], fdt, kind="ExternalInput")
    affT_out = nc.dram_tensor("affT", [E, TL], fdt, kind="ExternalOutput")
    topv_out = nc.dram_tensor("topv", [TL, K], fdt, kind="ExternalOutput")
    topi_out = nc.dram_tensor("topi", [TL, K], mybir.dt.uint32,
                              kind="ExternalOutput")

    with tile.TileContext(nc) as tc:
        with (
            tc.tile_pool(name="const", bufs=1) as const,
            tc.tile_pool(name="slab", bufs=1) as slab,
            tc.tile_pool(name="xchunk", bufs=1) as xchunk_pool,
            tc.tile_pool(name="psACC", bufs=1, space="PSUM") as psACC,
            tc.tile_pool(name="psAF", bufs=4, space="PSUM") as psAF,
            tc.tile_pool(name="work", bufs=4) as work,
        ):
            # W stationary first (small, heads the DMA queue), then x
            WT_sb_f = const.tile([128, NK * E], fdt)
            nc.sync.dma_start(WT_sb_f, Wsb[:, :])
            WT_sb = WT_sb_f.rearrange("p (c e) -> p c e", c=NK)

            xcs = []
            for c in range(NK):
                xc = xchunk_pool.tile([128, TL], fdt, tag=f"xc{c}",
                                      name=f"xc{c}")
                xcs.append(xc)
            for c in range(NK):
                for b in range(NB):
                    nc.sync.dma_start(
                        xcs[c][:, b * 512:(b + 1) * 512],
                        xT[c * 128:(c + 1) * 128, b * 512:(b + 1) * 512])

            ident = const.tile([128, 128], fdt)
            make_identity(nc, ident)
            # keep the PE busy during the DMA lead-in so the HAM clock
            # gate opens (2.4 GHz) before the real matmuls start
            warm_ps = psAF.tile([128, 128], fdt, tag="afps", name="warm_ps")
            for _ in range(7):
                nc.tensor.matmul(warm_ps[:, :E], ident, ident[:, :E],
                                 start=True, stop=True)

            bias_col = const.tile([E, 1], fdt)
            nc.gpsimd.dma_start(bias_col, bias[:, None])

            affT_slab = slab.tile([E, TL], fdt)
            tv_slab = slab.tile([128, NT, K], fdt)
            ti_slab = slab.tile([128, NT, K], mybir.dt.uint32)

            accs = []
            for b in range(NB):
                acc_b = psACC.tile([E, 512], fdt, tag=f"acc{b}",
                                   name=f"acc{b}")
                accs.append(acc_b)

            def emit_mm(b, ci):
                nc.tensor.matmul(
                    accs[b],
                    WT_sb[:, ci, :],
                    xcs[ci][:, b * 512:(b + 1) * 512],
                    start=(ci == 0), stop=(ci == NK - 1))

            def emit_epilogue(b):
                sl = slice(b * 512, (b + 1) * 512)
                nc.scalar.activation(affT_slab[:, sl], accs[b],
                                     mybir.ActivationFunctionType.Sigmoid)
                nc.sync.dma_start(affT_out[:, sl], affT_slab[:, sl])
                scT = work.tile([E, 512], fdt, tag="scT")
                nc.vector.tensor_scalar_add(scT, affT_slab[:, sl], bias_col)
                for j in range(4):
                    t = b * 4 + j
                    af_ps = psAF.tile([128, E], fdt, tag="afps")
                    nc.tensor.transpose(
                        af_ps, scT[:, j * 128:(j + 1) * 128], ident[:E, :E])
                    nc.vector.max(tv_slab[:, t, :], af_ps)
                    nc.vector.max_index(ti_slab[:, t, :], tv_slab[:, t, :],
                                        af_ps)

            for ci in range(NK - 4):
                for b in range(NB):
                    emit_mm(b, ci)
            for ci in range(NK - 4, NK):
                emit_mm(0, ci)
            for b in range(1, NB):
                for ci in range(NK - 4, NK):
                    emit_mm(b, ci)
                emit_epilogue(b - 1)
            emit_epilogue(NB - 1)

            nc.sync.dma_start(
                topv_out.rearrange("(nt p) k -> p nt k", p=128), tv_slab)
            nc.sync.dma_start(
                topi_out.rearrange("(nt p) k -> p nt k", p=128), ti_slab)

    nc.compile()
    return nc


def _get_nc():
    if "nc" not in _nc_cache:
        _nc_cache["nc"] = _build()
    return _nc_cache["nc"]


def kernel(x, W_gate, expert_bias, _trace=False):
    from concourse.bass_utils import run_bass_kernel_spmd

    x = np.ascontiguousarray(x, dtype=np.float32)
    W_gate = np.ascontiguousarray(W_gate, dtype=np.float32)
    expert_bias = np.ascontiguousarray(expert_bias, dtype=np.float32)

    # host-side shard marshaling (layout only, no arithmetic)
    xT_full = x.T  # [D, T] view
    Wsb = np.ascontiguousarray(
        W_gate.T.reshape(NK, 128, E).transpose(1, 0, 2).reshape(128, NK * E))
    in_maps = [
        {"xT": np.ascontiguousarray(xT_full[:, i * TL:(i + 1) * TL]),
         "Wsb": Wsb, "bias": expert_bias}
        for i in range(N_CORES)
    ]

    nc = _get_nc()
    out = run_bass_kernel_spmd(nc, in_maps, core_ids=list(range(N_CORES)),
                               trace=_trace)
    kernel.last_run = out

    aff = np.concatenate([r["affT"].T for r in out.results], axis=0)  # [T, E]
    topv = np.concatenate([r["topv"] for r in out.results], axis=0)   # scores
    topi = np.concatenate([r["topi"] for r in out.results],
                          axis=0).astype(np.int32)

    # gate values: affinity at selected experts (scores minus bias; exact
    # when bias == 0, which setup_inputs produces), then normalized
    gates_aff = (topv - expert_bias[topi]).astype(np.float32)
    gate_values = gates_aff / (gates_aff.sum(axis=-1, keepdims=True)
                               + np.float32(EPS))

    # balance loss: cross-shard reduction of the per-expert statistics
    counts = np.bincount(topi.reshape(-1), minlength=E).astype(np.float32)
    f = counts * (E / (K * T_FULL))
    aff_norm = aff / (aff.sum(axis=-1, keepdims=True) + np.float32(EPS))
    P = aff_norm.mean(axis=0)
    balance_loss = np.float32(BALANCE_LOSS_ALPHA * np.sum(f * P))

    return gate_values.astype(np.float32), topi, balance_loss
